# revision 5
# baseline (speedup 1.0000x reference)
"""Multi-head attention (B=2, T=2048, D=1024, H=16) on 8 Trainium2 NeuronCores.

Sharding: core c handles batch b = c//4 and heads 4*(c%4) .. 4*(c%4)+3.
Each core computes its 4 heads' att/k/v slices plus a partial y (its 256
columns of the concat-head dim through Wp); the host sums the 4 partial y
per batch and reassembles the full outputs.

Returns (y, att, k, v) matching the reference module.
"""

import sys

import numpy as np

if "/opt/trn_rl_repo" not in sys.path:
    sys.path.insert(0, "/opt/trn_rl_repo")

B, T, D = 2, 2048, 1024
H, HD = 16, 64
NCORES = 8
HPC = 4                 # heads per core
HC = HPC * HD           # 256: per-core slice of the concat-head dim
SCALE = 1.0 / 32.0      # 1/sqrt(D)
NEG = -1.0e30

TI = T // 128           # 16 row-tiles of 128
NCI = T // 512          # 4 row-chunks of 512


def build_nc():
    import concourse.bass as bass  # noqa: F401
    import concourse.mybir as mybir
    import concourse.tile as tile
    from concourse import bacc

    fp32 = mybir.dt.float32
    AF = mybir.ActivationFunctionType

    nc = bacc.Bacc(None, target_bir_lowering=False)

    # ---- DRAM I/O ----
    qt_d = nc.dram_tensor("qt", [D, T], fp32, kind="ExternalInput")    # Q[b].T
    kt_d = nc.dram_tensor("kt", [D, T], fp32, kind="ExternalInput")    # K[b].T
    vt_d = nc.dram_tensor("vt", [D, T], fp32, kind="ExternalInput")    # V[b].T
    wq_d = nc.dram_tensor("wqt", [D, HC], fp32, kind="ExternalInput")  # Wq[rows].T
    wk_d = nc.dram_tensor("wkt", [D, HC], fp32, kind="ExternalInput")
    wv_d = nc.dram_tensor("wvt", [D, HC], fp32, kind="ExternalInput")
    wp_d = nc.dram_tensor("wpt", [HC, D], fp32, kind="ExternalInput")  # Wp[:,cols].T
    maskd_d = nc.dram_tensor("maskd", [128, 128], fp32, kind="ExternalInput")
    m896_d = nc.dram_tensor("m896", [128, 896], fp32, kind="ExternalInput")
    ident_d = nc.dram_tensor("ident", [128, 128], fp32, kind="ExternalInput")

    att_d = nc.dram_tensor("att", [HPC, T, T], fp32, kind="ExternalOutput")
    ko_d = nc.dram_tensor("ko", [HPC, T, HD], fp32, kind="ExternalOutput")
    vo_d = nc.dram_tensor("vo", [HPC, T, HD], fp32, kind="ExternalOutput")
    yo_d = nc.dram_tensor("yo", [T, D], fp32, kind="ExternalOutput")

    with tile.TileContext(nc) as tc:
        with (
            tc.tile_pool(name="consts", bufs=1) as cpool,
            tc.tile_pool(name="weights", bufs=1) as wpool,
            tc.tile_pool(name="acts", bufs=1) as apool,
            tc.tile_pool(name="stream", bufs=2) as spool,
            tc.tile_pool(name="et", bufs=18) as etpool,
            tc.tile_pool(name="rows", bufs=3) as rowpool,
            tc.tile_pool(name="small", bufs=16) as smallpool,
            tc.tile_pool(name="ycat", bufs=8) as ycpool,
            tc.tile_pool(name="ymisc", bufs=4) as ympool,
            tc.tile_pool(name="yout", bufs=2) as yopool,
            tc.tile_pool(name="kout", bufs=4) as kopool,
            tc.tile_pool(name="psA", bufs=2, space="PSUM") as psA,   # proj/s-rows
            tc.tile_pool(name="psB", bufs=2, space="PSUM") as psB,   # ET
            tc.tile_pool(name="psY", bufs=1, space="PSUM") as psY,   # (128,64) AV
            tc.tile_pool(name="psT", bufs=1, space="PSUM") as psT,   # transposes
            tc.tile_pool(name="psP", bufs=2, space="PSUM") as psP,   # out-proj
        ):
            # ---- constants ----
            maskd = cpool.tile([128, 128], fp32)
            nc.sync.dma_start(maskd[:], maskd_d[:])
            m896 = cpool.tile([128, 896], fp32)
            nc.sync.dma_start(m896[:], m896_d[:])
            ident = cpool.tile([128, 128], fp32)
            nc.sync.dma_start(ident[:], ident_d[:])

            # ---- weights: [128, 8, HC] with d-chunk as middle dim ----
            wq = wpool.tile([128, 8, HC], fp32)
            nc.sync.dma_start(wq[:], wq_d.rearrange("(n p) c -> p n c", p=128))
            wk = wpool.tile([128, 8, HC], fp32)
            nc.sync.dma_start(wk[:], wk_d.rearrange("(n p) c -> p n c", p=128))
            wv = wpool.tile([128, 8, HC], fp32)
            nc.sync.dma_start(wv[:], wv_d.rearrange("(n p) c -> p n c", p=128))
            wp = wpool.tile([128, 2, D], fp32)
            nc.sync.dma_start(wp[:], wp_d.rearrange("(n p) c -> p n c", p=128))

            # ---- persistent activations ----
            # qt_cat/kt_cat: [128, 2, T]; partition+mid = 256 head-cols
            qt_cat = apool.tile([128, 2, T], fp32)
            kt_cat = apool.tile([128, 2, T], fp32)
            # v_cat: [128, 16, HC]; (t within tile, t-tile, head-col)
            v_cat = apool.tile([128, TI, HC], fp32)

            # ============ Phase 1: QKV projections ============
            NW = 256  # streamed t-chunk width
            for name, src, w_sb, dst in (
                ("q", qt_d, wq, qt_cat),
                ("k", kt_d, wk, kt_cat),
            ):
                for n in range(T // NW):
                    s_in = spool.tile([128, 8, NW], fp32, tag="stream_in")
                    nc.sync.dma_start(
                        s_in[:],
                        src[:, n * NW:(n + 1) * NW].rearrange(
                            "(n p) t -> p n t", p=128
                        ),
                    )
                    for m in range(2):
                        ps = psA.tile([128, 512], fp32, tag="psA")
                        for d in range(8):
                            nc.tensor.matmul(
                                ps[:, :NW],
                                lhsT=w_sb[:, d, m * 128:(m + 1) * 128],
                                rhs=s_in[:, d, :],
                                start=(d == 0),
                                stop=(d == 7),
                            )
                        nc.scalar.activation(
                            dst[:, m, n * NW:(n + 1) * NW], ps[:, :NW], AF.Copy
                        )

            for n in range(T // NW):
                s_in = spool.tile([128, 8, NW], fp32, tag="stream_in")
                nc.sync.dma_start(
                    s_in[:],
                    vt_d[:, n * NW:(n + 1) * NW].rearrange("(n p) t -> p n t", p=128),
                )
                for tl in range(NW // 128):
                    tj = (n * NW) // 128 + tl
                    ps = psA.tile([128, 512], fp32, tag="psA")
                    for d in range(8):
                        nc.tensor.matmul(
                            ps[:, :HC],
                            lhsT=s_in[:, d, tl * 128:(tl + 1) * 128],
                            rhs=wv[:, d, :],
                            start=(d == 0),
                            stop=(d == 7),
                        )
                    nc.scalar.activation(v_cat[:, tj, :], ps[:, :HC], AF.Copy)
                    # v out: [t(128), h(4), hd(64)]
                    nc.sync.dma_start(
                        vo_d[:, tj * 128:(tj + 1) * 128, :].rearrange(
                            "h t e -> t h e"
                        ),
                        v_cat[:, tj, :].rearrange("p (h e) -> p h e", h=HPC),
                    )

            # ---- k output: transpose kt_cat blocks to (t, hd) ----
            for m in range(2):
                for tj in range(TI):
                    pst = psT.tile([128, 128], fp32, tag="psT")
                    nc.tensor.transpose(
                        pst[:], kt_cat[:, m, tj * 128:(tj + 1) * 128], ident[:]
                    )
                    ko_sb = kopool.tile([128, 128], fp32, tag="ko")
                    nc.vector.tensor_copy(ko_sb[:], pst[:])
                    nc.sync.dma_start(
                        ko_d[2 * m:2 * m + 2, tj * 128:(tj + 1) * 128, :].rearrange(
                            "h t e -> t h e"
                        ),
                        ko_sb.rearrange("p (h e) -> p h e", h=2),
                    )

            # ============ Phase 2: attention ============
            for ci in range(NCI):
                ycat = [ycpool.tile([128, HC], fp32, tag="ycat", name=f"ycat{_t}") for _t in range(4)]
                for h in range(HPC):
                    hp, ho = h // 2, (h % 2) * 64
                    ntj = 4 * ci + 4
                    # ---- ET tiles: exp(scores.T) for this i-chunk ----
                    et_tiles = []
                    for tj in range(ntj):
                        ps = psB.tile([128, 512], fp32, tag="psB")
                        nc.tensor.matmul(
                            ps[:],
                            lhsT=kt_cat[ho:ho + 64, hp, tj * 128:(tj + 1) * 128],
                            rhs=qt_cat[ho:ho + 64, hp, ci * 512:(ci + 1) * 512],
                            start=True,
                            stop=True,
                        )
                        s = tj - 4 * ci
                        if s >= 0:
                            off = (3 - s) * 128
                            nc.vector.tensor_add(
                                ps[:], ps[:], m896[:, off:off + 512]
                            )
                        et = etpool.tile([128, 512], fp32, tag="et")
                        nc.scalar.activation(et[:], ps[:], AF.Exp, scale=SCALE)
                        et_tiles.append(et)

                    for tl in range(4):
                        ti = 4 * ci + tl
                        wrow = (ti + 1) * 128
                        nchunks = (wrow + 511) // 512
                        # ---- scores rows + softmax + att out ----
                        erow = rowpool.tile([128, T], fp32, tag="erow")
                        acc = smallpool.tile([128, 4], fp32, tag="acc")
                        for c in range(nchunks):
                            w = min(512, wrow - c * 512)
                            ps = psA.tile([128, 512], fp32, tag="psA")
                            nc.tensor.matmul(
                                ps[:, :w],
                                lhsT=qt_cat[ho:ho + 64, hp, ti * 128:(ti + 1) * 128],
                                rhs=kt_cat[ho:ho + 64, hp, c * 512:c * 512 + w],
                                start=True,
                                stop=True,
                            )
                            if c == nchunks - 1:
                                nc.vector.tensor_add(
                                    ps[:, w - 128:w], ps[:, w - 128:w], maskd[:]
                                )
                            nc.scalar.activation(
                                erow[:, c * 512:c * 512 + w],
                                ps[:, :w],
                                AF.Exp,
                                scale=SCALE,
                                accum_out=acc[:, c:c + 1],
                            )
                        den = smallpool.tile([128, 1], fp32, tag="den")
                        nc.vector.reduce_sum(
                            den[:], acc[:, :nchunks], axis=mybir.AxisListType.X
                        )
                        r = smallpool.tile([128, 1], fp32, tag="r")
                        nc.vector.reciprocal(r[:], den[:])
                        nc.vector.tensor_scalar_mul(
                            erow[:, :wrow], erow[:, :wrow], r[:]
                        )
                        nc.sync.dma_start(
                            att_d[h, ti * 128:(ti + 1) * 128, 0:wrow],
                            erow[:, :wrow],
                        )
                        # ---- AV: yU[i,hd] = sum_j ET[j,i] * v[j,hd] ----
                        psy = psY.tile([128, 64], fp32, tag="psY")
                        for tj in range(ti + 1):
                            nc.tensor.matmul(
                                psy[:],
                                lhsT=et_tiles[tj][:, tl * 128:(tl + 1) * 128],
                                rhs=v_cat[:, tj, h * 64:(h + 1) * 64],
                                start=(tj == 0),
                                stop=(tj == ti),
                            )
                        nc.scalar.activation(
                            ycat[tl][:, h * 64:(h + 1) * 64],
                            psy[:],
                            AF.Copy,
                            scale=r[:],
                        )

                # ---- output projection for this i-chunk ----
                for tl in range(4):
                    ti = 4 * ci + tl
                    yt = []
                    for c2 in range(2):
                        pst = psT.tile([128, 128], fp32, tag="psT")
                        nc.tensor.transpose(
                            pst[:], ycat[tl][:, c2 * 128:(c2 + 1) * 128], ident[:]
                        )
                        ycT = ympool.tile([128, 128], fp32, tag="ycT")
                        nc.vector.tensor_copy(ycT[:], pst[:])
                        yt.append(ycT)
                    yp = yopool.tile([128, D], fp32, tag="yp")
                    for dc in range(2):
                        psp = psP.tile([128, 512], fp32, tag="psP")
                        for c2 in range(2):
                            nc.tensor.matmul(
                                psp[:],
                                lhsT=yt[c2][:],
                                rhs=wp[:, c2, dc * 512:(dc + 1) * 512],
                                start=(c2 == 0),
                                stop=(c2 == 1),
                            )
                        nc.vector.tensor_copy(yp[:, dc * 512:(dc + 1) * 512], psp[:])
                    nc.sync.dma_start(yo_d[ti * 128:(ti + 1) * 128, :], yp[:])

    nc.compile()
    return nc


def _host_consts():
    i = np.arange(128)
    maskd = np.where(i[None, :] <= i[:, None], 0.0, NEG).astype(np.float32)
    # [Z(384) | T(128) | O(384)]: Z=kill, T=keep upper-incl-diag, O=keep
    m896 = np.zeros((128, 896), np.float32)
    m896[:, :384] = NEG
    tri = np.where(i[None, :] >= i[:, None], 0.0, NEG).astype(np.float32)
    m896[:, 384:512] = tri
    ident = np.eye(128, dtype=np.float32)
    return maskd, m896, ident


_NC_CACHE = {}


def kernel(Q, K, V, Wq, Wk, Wv, Wp):
    from concourse.bass_utils import run_bass_kernel_spmd

    Q = np.asarray(Q, np.float32)
    K = np.asarray(K, np.float32)
    V = np.asarray(V, np.float32)
    Wq = np.asarray(Wq, np.float32)
    Wk = np.asarray(Wk, np.float32)
    Wv = np.asarray(Wv, np.float32)
    Wp = np.asarray(Wp, np.float32)

    if "nc" not in _NC_CACHE:
        _NC_CACHE["nc"] = build_nc()
    nc = _NC_CACHE["nc"]

    maskd, m896, ident = _host_consts()
    qt = [np.ascontiguousarray(Q[b].T) for b in range(B)]
    kt = [np.ascontiguousarray(K[b].T) for b in range(B)]
    vt = [np.ascontiguousarray(V[b].T) for b in range(B)]

    in_maps = []
    for c in range(NCORES):
        b, g = c // 4, c % 4
        r0 = g * HC
        in_maps.append({
            "qt": qt[b],
            "kt": kt[b],
            "vt": vt[b],
            "wqt": np.ascontiguousarray(Wq[r0:r0 + HC, :].T),
            "wkt": np.ascontiguousarray(Wk[r0:r0 + HC, :].T),
            "wvt": np.ascontiguousarray(Wv[r0:r0 + HC, :].T),
            "wpt": np.ascontiguousarray(Wp[:, r0:r0 + HC].T),
            "maskd": maskd,
            "m896": m896,
            "ident": ident,
        })

    import os
    trace = bool(os.environ.get("BASS_TRACE"))
    tmpdir = os.environ.get("BASS_TRACE_DIR") or None
    if tmpdir:
        os.makedirs(tmpdir, exist_ok=True)
    res = run_bass_kernel_spmd(
        nc, in_maps, core_ids=list(range(NCORES)), trace=trace, tmpdir=tmpdir
    )
    _NC_CACHE["last_results"] = res
    outs = res.results

    y = np.zeros((B, T, D), np.float32)
    att = np.empty((B, H, T, T), np.float32)
    k = np.empty((B, H, T, HD), np.float32)
    v = np.empty((B, H, T, HD), np.float32)
    for c in range(NCORES):
        b, g = c // 4, c % 4
        hs = slice(g * HPC, (g + 1) * HPC)
        att[b, hs] = outs[c]["att"]
        k[b, hs] = outs[c]["ko"]
        v[b, hs] = outs[c]["vo"]
        y[b] += outs[c]["yo"]
    return (y, att, k, v)


if __name__ == "__main__":
    nc = build_nc()
    print("build ok:", len(nc.m.functions[0].allocations), "allocations")


# revision 13
# speedup vs baseline: 1.4236x; 1.4236x over previous
"""Multi-head attention (B=2, T=2048, D=1024, H=16) on 8 Trainium2 NeuronCores.

Sharding: core c handles batch b = c//4 and heads 4*(c%4) .. 4*(c%4)+3.
Each core computes its 4 heads' att/k/v slices plus a partial y (its 256
columns of the concat-head dim through Wp); the host sums the 4 partial y
per batch and reassembles the full outputs.

All matmuls run in fp32r (single-pass PE mode, ~2e-4 relative error);
fp32r operands are produced by casting DMAs (gpsimd) or engine copies.
Attention probabilities are written once per row-strip; the strictly-upper
causal triangle is never written (output buffers are pre-zeroed by the
runtime on both the native and PJRT paths).

Returns (y, att, k, v) matching the reference module.
"""

import sys

import numpy as np

if "/opt/trn_rl_repo" not in sys.path:
    sys.path.insert(0, "/opt/trn_rl_repo")

B, T, D = 2, 2048, 1024
H, HD = 16, 64
NCORES = 8
HPC = 4                 # heads per core
HC = HPC * HD           # 256: per-core slice of the concat-head dim
SCALE = 1.0 / 32.0      # 1/sqrt(D)
NEG = -1.0e30

TI = T // 128           # 16 row-tiles of 128
NCI = T // 512          # 4 row-chunks of 512


def build_nc():
    import concourse.bass as bass  # noqa: F401
    import concourse.mybir as mybir
    import concourse.tile as tile
    from concourse import bacc

    fp32 = mybir.dt.float32
    fp32r = mybir.dt.float32r
    AF = mybir.ActivationFunctionType

    nc = bacc.Bacc(None, target_bir_lowering=False)

    # ---- DRAM I/O ----
    qt_d = nc.dram_tensor("qt", [D, T], fp32, kind="ExternalInput")    # Q[b].T
    kt_d = nc.dram_tensor("kt", [D, T], fp32, kind="ExternalInput")    # K[b].T
    vt_d = nc.dram_tensor("vt", [D, T], fp32, kind="ExternalInput")    # V[b].T
    wq_d = nc.dram_tensor("wqt", [D, HC], fp32, kind="ExternalInput")  # Wq[rows].T
    wk_d = nc.dram_tensor("wkt", [D, HC], fp32, kind="ExternalInput")
    wv_d = nc.dram_tensor("wvt", [D, HC], fp32, kind="ExternalInput")
    wp_d = nc.dram_tensor("wpt", [HC, D], fp32, kind="ExternalInput")  # Wp[:,cols].T
    maskd_d = nc.dram_tensor("maskd", [128, 128], fp32, kind="ExternalInput")
    m896_d = nc.dram_tensor("m896", [128, 896], fp32, kind="ExternalInput")
    ident_d = nc.dram_tensor("ident", [128, 128], fp32, kind="ExternalInput")

    att_d = nc.dram_tensor("att", [HPC, T, T], fp32, kind="ExternalOutput")
    ko_d = nc.dram_tensor("ko", [HPC, T, HD], fp32, kind="ExternalOutput")
    vo_d = nc.dram_tensor("vo", [HPC, T, HD], fp32, kind="ExternalOutput")
    yo_d = nc.dram_tensor("yo", [T, D], fp32, kind="ExternalOutput")

    with tile.TileContext(nc) as tc:
        with (
            tc.tile_pool(name="consts", bufs=1) as cpool,
            tc.tile_pool(name="weights", bufs=1) as wpool,
            tc.tile_pool(name="acts", bufs=1) as apool,
            tc.tile_pool(name="stream", bufs=2) as spool,
            tc.tile_pool(name="et", bufs=17) as etpool,
            tc.tile_pool(name="rows", bufs=2) as rowpool,
            tc.tile_pool(name="small", bufs=16) as smallpool,
            tc.tile_pool(name="rbp", bufs=2) as rbpool,
            tc.tile_pool(name="yct", bufs=8) as ycpool,
            tc.tile_pool(name="yout", bufs=2) as yopool,
            tc.tile_pool(name="kout", bufs=4) as kopool,
            tc.tile_pool(name="psA", bufs=2, space="PSUM") as psA,   # proj/s-rows
            tc.tile_pool(name="psB", bufs=2, space="PSUM") as psB,   # ET
            tc.tile_pool(name="psY", bufs=1, space="PSUM") as psY,   # yT accum
            tc.tile_pool(name="psT", bufs=1, space="PSUM") as psT,   # transposes
            tc.tile_pool(name="psP", bufs=2, space="PSUM") as psP,   # out-proj
        ):
            # ---- constants ----
            maskd = cpool.tile([128, 128], fp32)
            nc.sync.dma_start(maskd[:], maskd_d[:])
            m896 = cpool.tile([128, 896], fp32)
            nc.sync.dma_start(m896[:], m896_d[:])
            ident = cpool.tile([128, 128], fp32)
            nc.sync.dma_start(ident[:], ident_d[:])
            identr = cpool.tile([128, 128], fp32r)
            nc.gpsimd.dma_start(identr[:], ident_d[:])

            # ---- weights (fp32r via casting DMA): [128, 8|2, cols] ----
            wq = wpool.tile([128, 8, HC], fp32r)
            nc.gpsimd.dma_start(wq[:], wq_d.rearrange("(n p) c -> p n c", p=128))
            wk = wpool.tile([128, 8, HC], fp32r)
            nc.gpsimd.dma_start(wk[:], wk_d.rearrange("(n p) c -> p n c", p=128))
            wv = wpool.tile([128, 8, HC], fp32r)
            nc.gpsimd.dma_start(wv[:], wv_d.rearrange("(n p) c -> p n c", p=128))
            wp = wpool.tile([64, 4, D], fp32r)
            nc.gpsimd.dma_start(wp[:], wp_d.rearrange("(n p) c -> p n c", p=64))

            # ---- persistent activations (fp32r) ----
            qt_cat = apool.tile([128, 2, T], fp32r)
            kt_cat = apool.tile([128, 2, T], fp32r)
            v_cat = apool.tile([128, TI, HC], fp32r)

            # ============ Phase 1: QKV projections ============
            NW = 256  # streamed t-chunk width
            for src, w_sb, dst in ((qt_d, wq, qt_cat), (kt_d, wk, kt_cat)):
                for n in range(T // NW):
                    s_in = spool.tile([128, 8, NW], fp32r, tag="stream_in")
                    nc.gpsimd.dma_start(
                        s_in[:],
                        src[:, n * NW:(n + 1) * NW].rearrange(
                            "(n p) t -> p n t", p=128
                        ),
                    )
                    for m in range(2):
                        ps = psA.tile([128, 512], fp32, tag="psA")
                        for d in range(8):
                            nc.tensor.matmul(
                                ps[:, :NW],
                                lhsT=w_sb[:, d, m * 128:(m + 1) * 128],
                                rhs=s_in[:, d, :],
                                start=(d == 0),
                                stop=(d == 7),
                            )
                        nc.scalar.activation(
                            dst[:, m, n * NW:(n + 1) * NW], ps[:, :NW], AF.Copy
                        )

            for n in range(T // NW):
                s_in = spool.tile([128, 8, NW], fp32r, tag="stream_in")
                nc.gpsimd.dma_start(
                    s_in[:],
                    vt_d[:, n * NW:(n + 1) * NW].rearrange("(n p) t -> p n t", p=128),
                )
                for tl in range(NW // 128):
                    tj = (n * NW) // 128 + tl
                    ps = psA.tile([128, 512], fp32, tag="psA")
                    for d in range(8):
                        nc.tensor.matmul(
                            ps[:, :HC],
                            lhsT=s_in[:, d, tl * 128:(tl + 1) * 128],
                            rhs=wv[:, d, :],
                            start=(d == 0),
                            stop=(d == 7),
                        )
                    nc.scalar.activation(v_cat[:, tj, :], ps[:, :HC], AF.Copy)
                    vstage = kopool.tile([128, HC], fp32, tag="vstage")
                    nc.vector.tensor_copy(vstage[:], ps[:, :HC])
                    nc.sync.dma_start(
                        vo_d[:, tj * 128:(tj + 1) * 128, :].rearrange(
                            "h t e -> t h e"
                        ),
                        vstage.rearrange("p (h e) -> p h e", h=HPC),
                    )

            # ---- k output: transpose kt_cat blocks to (t, hd) ----
            for m in range(2):
                for tj in range(TI):
                    pst = psT.tile([128, 128], fp32r, tag="psT")
                    nc.tensor.transpose(
                        pst[:], kt_cat[:, m, tj * 128:(tj + 1) * 128], identr[:]
                    )
                    ko_sb = kopool.tile([128, 128], fp32, tag="ko")
                    nc.vector.tensor_copy(ko_sb[:], pst[:])
                    nc.sync.dma_start(
                        ko_d[2 * m:2 * m + 2, tj * 128:(tj + 1) * 128, :].rearrange(
                            "h t e -> t h e"
                        ),
                        ko_sb.rearrange("p (h e) -> p h e", h=2),
                    )

            # ============ Phase 2: attention ============
            for ci in range(NCI):
                ycatT = [
                    ycpool.tile([64, 512], fp32r, tag="ycatT", name=f"ycatT{_h}")
                    for _h in range(HPC)
                ]
                for h in range(HPC):
                    hp, ho = h // 2, (h % 2) * 64
                    ntj = 4 * ci + 4
                    # ---- ET tiles: exp(scores.T) for this i-chunk ----
                    et_tiles = []
                    for tj in range(ntj):
                        ps = psB.tile([128, 512], fp32, tag="psB")
                        nc.tensor.matmul(
                            ps[:],
                            lhsT=kt_cat[ho:ho + 64, hp, tj * 128:(tj + 1) * 128],
                            rhs=qt_cat[ho:ho + 64, hp, ci * 512:(ci + 1) * 512],
                            start=True,
                            stop=True,
                        )
                        s = tj - 4 * ci
                        if s >= 0:
                            off = (3 - s) * 128
                            nc.vector.tensor_add(
                                ps[:], ps[:], m896[:, off:off + 512]
                            )
                        et = etpool.tile([128, 512], fp32r, tag="et")
                        nc.scalar.activation(et[:], ps[:], AF.Exp, scale=SCALE)
                        et_tiles.append(et)

                    r_row = rbpool.tile([1, 512], fp32, tag="r_row")
                    for tl in range(4):
                        ti = 4 * ci + tl
                        wrow = (ti + 1) * 128
                        nchunks = (wrow + 511) // 512
                        # ---- scores rows + softmax + att out ----
                        erow = rowpool.tile([128, T], fp32, tag="erow")
                        acc = smallpool.tile([128, 4], fp32, tag="acc")
                        for c in range(nchunks):
                            w = min(512, wrow - c * 512)
                            ps = psA.tile([128, 512], fp32, tag="psA")
                            nc.tensor.matmul(
                                ps[:],
                                lhsT=qt_cat[ho:ho + 64, hp, ti * 128:(ti + 1) * 128],
                                rhs=kt_cat[ho:ho + 64, hp, c * 512:(c + 1) * 512],
                                start=True,
                                stop=True,
                            )
                            if c == nchunks - 1:
                                nc.vector.tensor_add(
                                    ps[:, w - 128:w], ps[:, w - 128:w], maskd[:]
                                )
                            nc.scalar.activation(
                                erow[:, c * 512:c * 512 + w],
                                ps[:, :w],
                                AF.Exp,
                                scale=SCALE,
                                accum_out=acc[:, c:c + 1],
                            )
                        den = smallpool.tile([128, 1], fp32, tag="den")
                        nc.vector.reduce_sum(
                            den[:], acc[:, :nchunks], axis=mybir.AxisListType.X
                        )
                        r = smallpool.tile([128, 1], fp32, tag="r")
                        nc.vector.reciprocal(r[:], den[:])
                        nc.vector.tensor_scalar_mul(
                            erow[:, :wrow], erow[:, :wrow], r[:]
                        )
                        nc.sync.dma_start(
                            att_d[h, ti * 128:(ti + 1) * 128, 0:wrow],
                            erow[:, :wrow],
                        )
                        # r (128,1) -> r_row[0, tl*128:+128]
                        psr = psT.tile([128, 128], fp32, tag="psT")
                        nc.tensor.transpose(psr[:1, :], r[:], ident[:])
                        nc.vector.tensor_copy(
                            r_row[:, tl * 128:(tl + 1) * 128], psr[:1, :]
                        )

                    # ---- AV: yT[hd,i] = sum_j v[j,hd] ET[j,i], then * r ----
                    rb = rbpool.tile([64, 512], fp32, tag="rb")
                    nc.gpsimd.partition_broadcast(rb[:], r_row[:])
                    psy = psY.tile([64, 512], fp32, tag="psY")
                    for tj in range(ntj):
                        nc.tensor.matmul(
                            psy[:],
                            lhsT=v_cat[:, tj, h * 64:(h + 1) * 64],
                            rhs=et_tiles[tj][:],
                            start=(tj == 0),
                            stop=(tj == ntj - 1),
                        )
                    nc.vector.tensor_mul(ycatT[h][:], psy[:], rb[:])

                # ---- output projection for this i-chunk ----
                for tl in range(4):
                    ti = 4 * ci + tl
                    yp = yopool.tile([128, D], fp32, tag="yp")
                    for dc in range(2):
                        psp = psP.tile([128, 512], fp32, tag="psP")
                        for hq in range(HPC):
                            nc.tensor.matmul(
                                psp[:],
                                lhsT=ycatT[hq][:, tl * 128:(tl + 1) * 128],
                                rhs=wp[:, hq, dc * 512:(dc + 1) * 512],
                                start=(hq == 0),
                                stop=(hq == 3),
                            )
                        nc.vector.tensor_copy(yp[:, dc * 512:(dc + 1) * 512], psp[:])
                    nc.sync.dma_start(yo_d[ti * 128:(ti + 1) * 128, :], yp[:])

    nc.compile()
    return nc


def _host_consts():
    i = np.arange(128)
    maskd = np.where(i[None, :] <= i[:, None], 0.0, NEG).astype(np.float32)
    # [Z(384) | T(128) | O(384)]: Z=kill, T=keep upper-incl-diag, O=keep
    m896 = np.zeros((128, 896), np.float32)
    m896[:, :384] = NEG
    tri = np.where(i[None, :] >= i[:, None], 0.0, NEG).astype(np.float32)
    m896[:, 384:512] = tri
    ident = np.eye(128, dtype=np.float32)
    return maskd, m896, ident


_NC_CACHE = {}


def kernel(Q, K, V, Wq, Wk, Wv, Wp):
    from concourse.bass_utils import run_bass_kernel_spmd

    Q = np.asarray(Q, np.float32)
    K = np.asarray(K, np.float32)
    V = np.asarray(V, np.float32)
    Wq = np.asarray(Wq, np.float32)
    Wk = np.asarray(Wk, np.float32)
    Wv = np.asarray(Wv, np.float32)
    Wp = np.asarray(Wp, np.float32)

    if "nc" not in _NC_CACHE:
        _NC_CACHE["nc"] = build_nc()
    nc = _NC_CACHE["nc"]

    maskd, m896, ident = _host_consts()
    qt = [np.ascontiguousarray(Q[b].T) for b in range(B)]
    kt = [np.ascontiguousarray(K[b].T) for b in range(B)]
    vt = [np.ascontiguousarray(V[b].T) for b in range(B)]

    in_maps = []
    for c in range(NCORES):
        b, g = c // 4, c % 4
        r0 = g * HC
        in_maps.append({
            "qt": qt[b],
            "kt": kt[b],
            "vt": vt[b],
            "wqt": np.ascontiguousarray(Wq[r0:r0 + HC, :].T),
            "wkt": np.ascontiguousarray(Wk[r0:r0 + HC, :].T),
            "wvt": np.ascontiguousarray(Wv[r0:r0 + HC, :].T),
            "wpt": np.ascontiguousarray(Wp[:, r0:r0 + HC].T),
            "maskd": maskd,
            "m896": m896,
            "ident": ident,
        })

    import os
    trace = bool(os.environ.get("BASS_TRACE"))
    tmpdir = os.environ.get("BASS_TRACE_DIR") or None
    if tmpdir:
        os.makedirs(tmpdir, exist_ok=True)
    res = run_bass_kernel_spmd(
        nc, in_maps, core_ids=list(range(NCORES)), trace=trace, tmpdir=tmpdir
    )
    _NC_CACHE["last_results"] = res
    outs = res.results

    y = np.zeros((B, T, D), np.float32)
    att = np.empty((B, H, T, T), np.float32)
    k = np.empty((B, H, T, HD), np.float32)
    v = np.empty((B, H, T, HD), np.float32)
    for c in range(NCORES):
        b, g = c // 4, c % 4
        hs = slice(g * HPC, (g + 1) * HPC)
        att[b, hs] = outs[c]["att"]
        k[b, hs] = outs[c]["ko"]
        v[b, hs] = outs[c]["vo"]
        y[b] += outs[c]["yo"]
    return (y, att, k, v)


if __name__ == "__main__":
    nc = build_nc()
    print("build ok:", len(nc.m.functions[0].allocations), "allocations")
